# revision 1
# baseline (speedup 1.0000x reference)
"""Trainium2 Bass kernel for nn_AttentionBlock (GroupNorm + MHA + residual).

Sharding: data-parallel over batch. B=16 images, 8 cores -> 2 images/core.
Each core computes the full attention block for its 2 images. No collectives.

Per-image pipeline (all on one NeuronCore):
  1. GroupNorm(32 groups) via per-channel bn_stats + PE group-sum matmul,
     broadcast back to channels with a tiny K=8 matmul.
  2. qkv projection with host-pre-transposed weights. q/k produced in
     [head_dim, seq] layout (lhsT = W^T tiles), v produced in [seq, head_dim]
     layout (lhsT = h) with an extra ones-column per head.
  3. scores^T[j,i] = k^T.T @ q^T per head (K=64, two heads row-packed in the
     128-wide PE array), exp via ScalarE with fused 1/8 scale (softmax max-
     subtraction skipped: |scores/8| < ~10, exp is exact to 2 ULP there).
  4. attn@v with v' stationary: res'^T[d,i] accumulated over key tiles; the
     ones-column yields the softmax denominator on partition 64 for free.
     Normalize via DVE reciprocal + K=1 PE broadcast matmul.
  5. out projection + residual add, store.
"""

import sys

sys.path.insert(0, "/opt/trn_rl_repo")

import numpy as np

import concourse.bacc as bacc
import concourse.bass as bass
import concourse.tile as tile
from concourse import mybir
from concourse.bass_utils import run_bass_kernel_spmd

F32 = mybir.dt.float32
F32R = mybir.dt.float32r
BF16 = mybir.dt.bfloat16
AF = mybir.ActivationFunctionType
OP = mybir.AluOpType

B, C, HH, WW = 16, 512, 32, 32
S = HH * WW            # 1024
G = 32                 # groups
GS = C // G            # 16 channels / group
NH, DK = 8, 64
P = 128
NCORES = 8
BPC = B // NCORES      # images per core
CT = C // P            # 4 channel tiles
EPS = 1e-5
GN = GS * S            # elements per group (16384)

LAST_RESULT = {}       # stash for test.py introspection


def build_nc(bpc=BPC, apply_gnwb=False, apply_qb=False, apply_kb=False,
             apply_vb=False, apply_ob=False, reps=1):
    nc = bacc.Bacc("TRN2", target_bir_lowering=False, debug=False,
                   enable_asserts=False)

    x_d = nc.dram_tensor("x", [bpc, C, S], F32, kind="ExternalInput").ap()
    wq_d = nc.dram_tensor("wq", [C, C], BF16, kind="ExternalInput").ap()
    wk_d = nc.dram_tensor("wk", [C, C], BF16, kind="ExternalInput").ap()
    wv_d = nc.dram_tensor("wv", [C, C], BF16, kind="ExternalInput").ap()
    wo_d = nc.dram_tensor("wo", [C, C], BF16, kind="ExternalInput").ap()
    bq_d = nc.dram_tensor("bq", [C], F32, kind="ExternalInput").ap()
    bk_d = nc.dram_tensor("bk", [C], F32, kind="ExternalInput").ap()
    bv_d = nc.dram_tensor("bv", [C], F32, kind="ExternalInput").ap()
    ob_d = nc.dram_tensor("ob", [C], F32, kind="ExternalInput").ap()
    gnw_d = nc.dram_tensor("gnw", [C], F32, kind="ExternalInput").ap()
    gnb_d = nc.dram_tensor("gnb", [C], F32, kind="ExternalInput").ap()
    sel_d = nc.dram_tensor("sel", [P, P // GS], F32, kind="ExternalInput").ap()
    bsel_d = nc.dram_tensor("bsel", [P // GS, P], F32,
                            kind="ExternalInput").ap()
    out_d = nc.dram_tensor("out", [bpc, C, S], F32, kind="ExternalOutput").ap()

    from contextlib import ExitStack
    with tile.TileContext(nc) as tc, ExitStack() as ctx:
        consts = ctx.enter_context(tc.tile_pool(name="consts", bufs=1))
        xpool = ctx.enter_context(tc.tile_pool(name="xpool", bufs=2))
        hpool = ctx.enter_context(tc.tile_pool(name="hpool", bufs=2))
        qkpool = ctx.enter_context(tc.tile_pool(name="qkpool", bufs=2))
        vpool = ctx.enter_context(tc.tile_pool(name="vpool", bufs=2))
        expool = ctx.enter_context(tc.tile_pool(name="expool", bufs=3))
        respool = ctx.enter_context(tc.tile_pool(name="respool", bufs=1))
        opool = ctx.enter_context(tc.tile_pool(name="opool", bufs=4))
        small = ctx.enter_context(tc.tile_pool(name="small", bufs=4))
        pj = ctx.enter_context(tc.tile_pool(name="pj", bufs=2, space="PSUM"))
        pscore = ctx.enter_context(
            tc.tile_pool(name="pscore", bufs=2, space="PSUM"))
        pres = ctx.enter_context(tc.tile_pool(name="pres", bufs=2, space="PSUM"))
        dpool = ctx.enter_context(tc.tile_pool(name="dpool", bufs=4,
                                               space="DRAM"))

        # ---- x prefetch (one-shot mode): x gates the groupnorm critical
        # path, so issue its DMAs before the 4MB of weight loads ----
        pre_x = []
        if reps == 1:
            for b in range(bpc):
                x_sb = xpool.tile([P, CT, S], F32, tag="x", name=f"prex_{b}")
                x_v = x_d[b].rearrange("(t p) s -> p t s", p=P)
                for ct in range(CT):
                    nc.sync.dma_start(x_sb[:, ct], x_v[:, ct])
                pre_x.append(x_sb)

        # ---- one-time constants ----
        # group-sum selector: sel[c, g] = 1/GS if c//GS == g  (per 128-tile)
        sel_st = consts.tile([P, P // GS], F32, tag="sel_st")
        nc.sync.dma_start(sel_st, sel_d)
        sel_sb = consts.tile([P, P // GS], F32, tag="sel")
        nc.vector.tensor_copy(sel_sb, sel_st)
        # broadcast selector: bsel[g, c] = 1 if c//GS == g
        bsel_st = consts.tile([P // GS, P], F32, tag="bsel_st")
        nc.sync.dma_start(bsel_st, bsel_d)
        bsel_sb = consts.tile([P // GS, P], F32, tag="bsel")
        nc.vector.tensor_copy(bsel_sb, bsel_st)
        ones1_sb = consts.tile([1, DK], BF16, tag="ones1")
        nc.vector.memset(ones1_sb, 1.0)
        eps_sb = consts.tile([P // GS, 1], F32, tag="eps")
        nc.vector.memset(eps_sb, EPS)

        wq_sb = consts.tile([P, CT, C], BF16, tag="wq")
        wk_sb = consts.tile([P, CT, C], BF16, tag="wk")
        wv_sb = consts.tile([P, CT, C], BF16, tag="wv")
        wo_sb = consts.tile([P, CT, C], BF16, tag="wo")
        for w_sb, w_d in ((wq_sb, wq_d), (wk_sb, wk_d), (wv_sb, wv_d),
                          (wo_sb, wo_d)):
            w_v = w_d.rearrange("(t p) j -> p t j", p=P)
            for ct in range(CT):
                nc.sync.dma_start(w_sb[:, ct], w_v[:, ct])

        bq_sb = consts.tile([P, CT], F32, tag="bq")
        bk_sb = consts.tile([P, CT], F32, tag="bk")
        ob_sb = consts.tile([P, CT], F32, tag="ob")
        for b_sb, b_d in ((bq_sb, bq_d), (bk_sb, bk_d), (ob_sb, ob_d)):
            nc.sync.dma_start(b_sb, b_d.rearrange("(t p) -> p t", p=P))
        gnw_sb = consts.tile([P, CT], F32, tag="gnw")
        gnb_sb = consts.tile([P, CT], F32, tag="gnb")
        nc.sync.dma_start(gnw_sb, gnw_d.rearrange("(t p) -> p t", p=P))
        nc.sync.dma_start(gnb_sb, gnb_d.rearrange("(t p) -> p t", p=P))
        # v bias broadcast to all 128 partitions (added along free dim)
        bv_sb = consts.tile([P, C], F32, tag="bv")
        nc.sync.dma_start(bv_sb, bass.AP(tensor=bv_d.tensor, offset=bv_d.offset,
                                         ap=[[0, P]] + list(bv_d.ap)))

        NG_T = P // GS  # 8 groups per channel-tile

        from contextlib import nullcontext
        loop_ctx = tc.For_i(0, reps, 1) if reps > 1 else nullcontext()
        with loop_ctx:
            for b in range(bpc):
                # ================= load x =================
                if pre_x:
                    x_sb = pre_x[b]
                else:
                    x_sb = xpool.tile([P, CT, S], F32, tag="x")
                    x_v = x_d[b].rearrange("(t p) s -> p t s", p=P)
                    for ct in range(CT):
                        nc.sync.dma_start(x_sb[:, ct], x_v[:, ct])

                # ================= GroupNorm =================
                stats_all = small.tile([P, CT, 2], F32, tag="stats")
                for t in range(CT):
                    st6 = small.tile([P, 2, 6], F32, tag="bnst")
                    nc.vector.bn_stats(st6[:, 0], x_sb[:, t, 0:512])
                    nc.vector.bn_stats(st6[:, 1], x_sb[:, t, 512:1024])
                    mv = small.tile([P, 2], F32, tag="mv")
                    nc.vector.bn_aggr(mv, st6)
                    # stats_all[:,t,0] = mean_c ; stats_all[:,t,1] = E[x^2]_c
                    nc.vector.tensor_copy(stats_all[:, t, 0:1], mv[:, 0:1])
                    m2 = small.tile([P, 1], F32, tag="m2")
                    nc.vector.tensor_mul(m2, mv[:, 0:1], mv[:, 0:1])
                    nc.vector.tensor_add(stats_all[:, t, 1:2], m2, mv[:, 1:2])

                gsum_ps = pj.tile([P, 512], F32, tag="proj", name="gsum")
                nc.tensor.matmul(gsum_ps[:NG_T, :CT * 2], sel_sb,
                                 stats_all.rearrange("p t c -> p (t c)"),
                                 start=True, stop=True)
                gs_sb = small.tile([NG_T, CT, 2], F32, tag="gs")
                nc.vector.tensor_copy(
                    gs_sb, gsum_ps[:NG_T, :CT * 2].rearrange("p (t c) -> p t c", c=2))
                # var_g = E[x^2]_g - mean_g^2 ; rstd = 1/sqrt(var+eps)
                m2g = small.tile([NG_T, CT], F32, tag="m2g")
                nc.vector.tensor_mul(m2g, gs_sb[:, :, 0], gs_sb[:, :, 0])
                var_g = small.tile([NG_T, CT], F32, tag="varg")
                nc.vector.tensor_tensor(var_g, gs_sb[:, :, 1], m2g, OP.subtract)
                lg_g = small.tile([NG_T, CT], F32, tag="lgg")
                nc.scalar.activation(lg_g, var_g, AF.Ln, bias=eps_sb)
                rstd_g = small.tile([NG_T, CT], F32, tag="rstdg")
                nc.scalar.activation(rstd_g, lg_g, AF.Exp, scale=-0.5)
                pk2 = small.tile([NG_T, CT, 2], F32, tag="pk2")
                nc.vector.tensor_copy(pk2[:, :, 0], gs_sb[:, :, 0])
                nc.vector.tensor_copy(pk2[:, :, 1], rstd_g)

                h_sb = hpool.tile([P, CT, S], BF16, tag="h")
                for t in range(CT):
                    bc_ps = pj.tile([P, 512], F32, tag="proj", name="bcps")
                    nc.tensor.matmul(bc_ps[:, :2], bsel_sb, pk2[:, t, :],
                                     start=True, stop=True)
                    bc_sb = small.tile([P, 2], F32, tag="gnbc")
                    nc.vector.tensor_copy(bc_sb, bc_ps[:, :2])
                    nc.vector.tensor_scalar(
                        h_sb[:, t, :], x_sb[:, t, :],
                        scalar1=bc_sb[:, 0:1], scalar2=bc_sb[:, 1:2],
                        op0=OP.subtract, op1=OP.mult)
                    if apply_gnwb:
                        nc.vector.tensor_scalar(
                            h_sb[:, t, :], h_sb[:, t, :],
                            scalar1=gnw_sb[:, t:t + 1], scalar2=gnb_sb[:, t:t + 1],
                            op0=OP.mult, op1=OP.add)

                # ================= qkv projection =================
                # q/k: out[j, s] = W[j,:] @ h[:, s]  (lhsT = W^T tile, rhs = h)
                qk_sb = qkpool.tile([P, 2 * CT, S], BF16, tag="qk")  # [0:4]=q [4:8]=k
                for (w_sb, base, b_sb, app) in ((wq_sb, 0, bq_sb, apply_qb),
                                                (wk_sb, CT, bk_sb, apply_kb)):
                    for jt in range(CT):
                        for ib in range(2):
                            ps = pj.tile([P, 512], F32, tag="proj")
                            for ct in range(CT):
                                nc.tensor.matmul(
                                    ps,
                                    w_sb[:, ct, jt * P:(jt + 1) * P],
                                    h_sb[:, ct, ib * 512:(ib + 1) * 512],
                                    start=(ct == 0), stop=(ct == CT - 1))
                            dst = qk_sb[:, base + jt, ib * 512:(ib + 1) * 512]
                            if app:
                                nc.vector.tensor_scalar(
                                    dst, ps, scalar1=b_sb[:, jt:jt + 1],
                                    scalar2=None, op0=OP.add)
                            else:
                                nc.vector.tensor_copy(dst, ps)

                # v: out[s, jv] = h[:, s].T @ Wv^T ; jv grouped 65/head (ones col)
                v_sb = vpool.tile([P, S // P, NH * (DK + 1)], BF16, tag="v")
                nc.vector.memset(v_sb[:, :, DK::DK + 1], 1.0)
                for st in range(S // P):
                    ps = pj.tile([P, 512], F32, tag="proj")
                    for ct in range(CT):
                        nc.tensor.matmul(
                            ps,
                            h_sb[:, ct, st * P:(st + 1) * P],
                            wv_sb[:, ct, :],
                            start=(ct == 0), stop=(ct == CT - 1))
                    dst = v_sb[:, st, :].rearrange(
                        "p (h e) -> p h e", e=DK + 1)[:, :, 0:DK]
                    src = ps.rearrange("p (h d) -> p h d", d=DK)
                    if apply_vb:
                        nc.vector.tensor_tensor(
                            dst, src, bv_sb.rearrange("p (h d) -> p h d", d=DK),
                            OP.add)
                    else:
                        nc.vector.tensor_copy(dst, src)

                # ================= attention =================
                res_sb = respool.tile([P, CT, S], BF16, tag="res")
                for pt in range(NH // 2):       # head pairs (2pt, 2pt+1)
                    ex_t = [expool.tile([P, S // P, S], BF16, tag="ex",
                                        name=f"ex_{b}_{pt}_{i}")
                            for i in range(2)]
                    for jt in range(S // P):    # key tile
                        pss = [pscore.tile([P, S], F32, tag="score",
                                           name=f"sc_{b}_{pt}_{jt}_{hp}")
                               for hp in range(2)]
                        # interleave the two heads' K=64 matmuls so adjacent
                        # PE instructions hit disjoint row-groups (rows 0-63
                        # vs 64-127) and overlap in the array
                        for ib in range(2):  # query block of 512
                            for hp in range(2):
                                pr = slice(hp * 64, hp * 64 + 64)
                                nc.tensor.matmul(
                                    pss[hp][:, ib * 512:(ib + 1) * 512],
                                    qk_sb[pr, CT + pt, jt * P:(jt + 1) * P],
                                    qk_sb[pr, pt, ib * 512:(ib + 1) * 512],
                                    start=True, stop=True)
                        # exp(q.k/8); scale fused into activation
                        for hp in range(2):
                            nc.scalar.activation(ex_t[hp][:, jt, :], pss[hp],
                                                 AF.Exp, scale=0.125)

                    for hp in range(2):
                        h_abs = 2 * pt + hp
                        ex_sb = ex_t[hp]
                        for ib in range(2):
                            rp = pres.tile([P, 512], F32, tag="res")
                            for jt in range(S // P):
                                nc.tensor.matmul(
                                    rp[:DK + 1],
                                    v_sb[:, jt,
                                         h_abs * (DK + 1):(h_abs + 1) * (DK + 1)],
                                    ex_sb[:, jt, ib * 512:(ib + 1) * 512],
                                    start=(jt == 0), stop=(jt == S // P - 1))
                            # normalize: res = res' * (1/den); den on partition 64
                            rec = small.tile([1, 512], BF16, tag="rec")
                            with nc.allow_low_precision(
                                    reason="softmax denom recip in bf16"):
                                nc.vector.reciprocal(rec, rp[DK:DK + 1, :])
                            bc_ps = pj.tile([P, 512], F32, tag="proj",
                                            name="bcps")
                            nc.tensor.matmul(bc_ps[:DK], ones1_sb, rec,
                                             start=True, stop=True)
                            bc_sb = small.tile([DK, 512], F32, tag="bcsb")
                            nc.vector.tensor_copy(bc_sb, bc_ps[:DK])
                            dst = res_sb[(h_abs % 2) * DK:(h_abs % 2) * DK + DK,
                                         h_abs // 2, ib * 512:(ib + 1) * 512]
                            nc.vector.tensor_mul(dst, rp[:DK], bc_sb)

                # ================= out projection + residual =================
                out_v = out_d[b].rearrange("(t p) s -> p t s", p=P)
                for ot in range(CT):
                    for ib in range(2):
                        ps = pj.tile([P, 512], F32, tag="proj")
                        for ct in range(CT):
                            nc.tensor.matmul(
                                ps,
                                wo_sb[:, ct, ot * P:(ot + 1) * P],
                                res_sb[:, ct, ib * 512:(ib + 1) * 512],
                                start=(ct == 0), stop=(ct == CT - 1))
                        o_sb = opool.tile([P, 512], F32, tag="ostage")
                        nc.vector.tensor_tensor(
                            o_sb, ps, x_sb[:, ot, ib * 512:(ib + 1) * 512], OP.add)
                        if apply_ob:
                            nc.vector.tensor_scalar(
                                o_sb, o_sb, scalar1=ob_sb[:, ot:ot + 1],
                                scalar2=None, op0=OP.add)
                        nc.sync.dma_start(out_v[:, ot, ib * 512:(ib + 1) * 512],
                                          o_sb)
    nc.finalize()
    return nc


def host_sel():
    ng_t = P // GS
    sel = np.zeros((P, ng_t), np.float32)
    bsel = np.zeros((ng_t, P), np.float32)
    for g in range(ng_t):
        sel[g * GS:(g + 1) * GS, g] = 1.0 / GS
        bsel[g, g * GS:(g + 1) * GS] = 1.0
    return sel, bsel


def host_prep(proj_w, proj_b, out_w):
    """Split + reorder projection weights; returns transposed [C_in, C_out]."""
    q_rows, k_rows = [], []
    for t in range(NH // 2):
        for hh in (2 * t, 2 * t + 1):
            q_rows += list(range(hh * 3 * DK, hh * 3 * DK + DK))
            k_rows += list(range(hh * 3 * DK + DK, hh * 3 * DK + 2 * DK))
    v_rows = [hh * 3 * DK + 2 * DK + d for hh in range(NH) for d in range(DK)]
    import ml_dtypes
    bf = ml_dtypes.bfloat16
    wq = np.ascontiguousarray(proj_w[q_rows, :].T).astype(bf)
    wk = np.ascontiguousarray(proj_w[k_rows, :].T).astype(bf)
    wv = np.ascontiguousarray(proj_w[v_rows, :].T).astype(bf)
    wo = np.ascontiguousarray(out_w.T).astype(bf)
    bq = np.ascontiguousarray(proj_b[q_rows])
    bk = np.ascontiguousarray(proj_b[k_rows])
    bv = np.ascontiguousarray(proj_b[v_rows])
    return wq, wk, wv, wo, bq, bk, bv


def kernel(x, gn_w, gn_b, proj_w, proj_b, out_w, out_b):
    x = np.asarray(x, dtype=np.float32)
    gn_w = np.asarray(gn_w, dtype=np.float32)
    gn_b = np.asarray(gn_b, dtype=np.float32)
    proj_w = np.asarray(proj_w, dtype=np.float32)
    proj_b = np.asarray(proj_b, dtype=np.float32)
    out_w = np.asarray(out_w, dtype=np.float32)
    out_b = np.asarray(out_b, dtype=np.float32)

    wq, wk, wv, wo, bq, bk, bv = host_prep(proj_w, proj_b, out_w)
    sel, bsel = host_sel()
    apply_gnwb = not (np.all(gn_w == 1.0) and np.all(gn_b == 0.0))
    apply_qb = bool(np.any(bq != 0.0))
    apply_kb = bool(np.any(bk != 0.0))
    apply_vb = bool(np.any(bv != 0.0))
    apply_ob = bool(np.any(out_b != 0.0))

    nc = build_nc(BPC, apply_gnwb, apply_qb, apply_kb, apply_vb, apply_ob)

    xr = x.reshape(B, C, S)
    in_maps = []
    for c in range(NCORES):
        in_maps.append({
            "x": np.ascontiguousarray(xr[c * BPC:(c + 1) * BPC]),
            "wq": wq, "wk": wk, "wv": wv, "wo": wo,
            "bq": bq, "bk": bk, "bv": bv, "ob": out_b,
            "gnw": gn_w, "gnb": gn_b, "sel": sel, "bsel": bsel,
        })

    import os
    trace = bool(int(os.environ.get("KERNEL_TRACE", "0")))
    r = run_bass_kernel_spmd(nc, in_maps, core_ids=list(range(NCORES)),
                             trace=trace)
    LAST_RESULT["results"] = r
    out = np.concatenate([r.results[c]["out"] for c in range(NCORES)], axis=0)
    return out.reshape(B, C, HH, WW).astype(np.float32)

